# revision 1
# baseline (speedup 1.0000x reference)
"""Power-STFT kernel for Trainium2 (8 NeuronCores, data-parallel over batch).

Computes, for x [32, 320000] and scalar lambd:
    x <- x - mean(x, axis=1)
    power-STFT (n_fft=1024, hop=320, periodic Hann, center reflect pad)
    out = log1p(lambd * power)   -> [32, 513, 1001] fp32

Strategy per core (4 batch samples per core):
  - Host prepares two reshaped copies of the reflect-padded signal so that
    time index mod 128 lies on the SBUF partition axis (slab A: offset 0 for
    even frames, slab B: offset 64 for odd frames; hop=320 => frame t starts
    at 128*(5v)+0 for t=2v and 128*(5v+2)+64 for t=2v+1).
  - Windowed DFT as fp32r matmuls (full PE rate; contraction = window
    position, 8 chunks of 128 on partitions; frames on the moving free dim
    via stride-5 APs). sqrt(lambd) is folded into the DFT matrices.
  - Mean removal is folded into the epilogue: STFT(x - mu) = STFT(x) -
    mu * colsum(W), applied as the per-partition bias of the ACT Square
    (power = (X + bias)^2), so no subtract pass over the signal is needed.
  - power = cos^2 + sin^2 (ACT Square x2 + GPSIMD add), log1p via ACT
    Ln(p + 1). The Nyquist bin (512) rides in the sin-chunk-0 zero column
    (sin k=0 == 0); bins 0/512 are fixed up with narrow 1-partition ops.
"""

import sys

sys.path.insert(0, "/opt/trn_rl_repo")

import numpy as np

import concourse.bacc as bacc
import concourse.bass as bass
import concourse.mybir as mybir
import concourse.tile as tile
from contextlib import ExitStack

N_FFT = 1024
HOP = 320
L = 320000
PAD = N_FFT // 2  # 512
LP = L + 2 * PAD  # 321024
B = 32
NCORES = 8
SPC = B // NCORES  # 4 samples per core
T = 1 + L // HOP  # 1001 frames
TEV = 501  # even frames t=0,2,...,1000
TOD = 500  # odd frames
NEV = 502  # padded even-frame count (fp32r moving free dim must be even)
NOD = 500
NBINS = 513
QCOLS = 2515  # slab columns (multiple of 5, covers col 5*501+7=2512)
TPAD = 1002  # out tile free dim (T padded even for parity interleave)

_f32 = mybir.dt.float32
_f32r = mybir.dt.float32r


def _build_module():
    nc = bacc.Bacc(None, target_bir_lowering=False, debug=False)

    xa_d = nc.dram_tensor("xa", [SPC, 128, QCOLS], _f32, kind="ExternalInput")
    xb_d = nc.dram_tensor("xb", [SPC, 128, QCOLS], _f32, kind="ExternalInput")
    wc_d = nc.dram_tensor("wc", [8, 128, 512], _f32, kind="ExternalInput")
    ws_d = nc.dram_tensor("ws", [8, 128, 512], _f32, kind="ExternalInput")
    nbars_d = nc.dram_tensor("nbars", [8, 128], _f32, kind="ExternalInput")
    eye8_d = nc.dram_tensor("eye8", [8, 8], _f32, kind="ExternalInput")
    o_d = nc.dram_tensor("o", [SPC, NBINS, T], _f32, kind="ExternalOutput")

    with tile.TileContext(nc) as tc:
        with ExitStack() as ctx:
            consts = ctx.enter_context(tc.tile_pool(name="consts", bufs=1))
            slabs = ctx.enter_context(tc.tile_pool(name="slabs", bufs=3))
            stats = ctx.enter_context(tc.tile_pool(name="stats", bufs=4))
            tmps = ctx.enter_context(tc.tile_pool(name="tmps", bufs=4))
            outs = ctx.enter_context(tc.tile_pool(name="outs", bufs=3))
            psums = ctx.enter_context(tc.tile_pool(name="psum", bufs=3, space="PSUM"))
            mupsum = ctx.enter_context(tc.tile_pool(name="mupsum", bufs=1, space="PSUM"))

            ones_col = consts.tile([128, 1], _f32)
            nc.vector.memset(ones_col, 1.0)
            ones_row = consts.tile([1, 128], _f32)
            nc.vector.memset(ones_row, 1.0)
            # sample-0 A slab first (mean chain + even-frame groups), then
            # weights, then the B slab (odd frames run later anyway)
            s0a = slabs.tile([128, QCOLS], _f32r, tag="ar")
            nc.sync.dma_start(out=s0a, in_=xa_d[0, :, :].bitcast(_f32r))

            wc_sb = consts.tile([128, 8, 512], _f32r)
            ws_sb = consts.tile([128, 8, 512], _f32r)
            for uh in range(2):
                usl = slice(4 * uh, 4 * uh + 4)
                nc.sync.dma_start(
                    out=wc_sb[:, usl, :],
                    in_=wc_d[usl, :, :].rearrange("u m k -> m u k").bitcast(_f32r),
                )
                nc.sync.dma_start(
                    out=ws_sb[:, usl, :],
                    in_=ws_d[usl, :, :].rearrange("u m k -> m u k").bitcast(_f32r),
                )
            s0b = slabs.tile([128, QCOLS], _f32r, tag="br")
            nc.sync.dma_start(out=s0b, in_=xb_d[0, :, :].bitcast(_f32r))
            first_slabs = {0: (s0a, s0b)}
            nbars_sb = consts.tile([8, 128], _f32)
            nc.sync.dma_start(out=nbars_sb, in_=nbars_d[:, :])
            eye8_sb = consts.tile([8, 8], _f32)
            nc.sync.dma_start(out=eye8_sb, in_=eye8_d[:, :])

            for s in range(SPC):
                if s in first_slabs:
                    ar, br = first_slabs[s]
                else:
                    ar = slabs.tile([128, QCOLS], _f32r, tag="ar")
                    nc.sync.dma_start(out=ar, in_=xa_d[s, :, :].bitcast(_f32r))
                    br = slabs.tile([128, QCOLS], _f32r, tag="br")
                    nc.sync.dma_start(out=br, in_=xb_d[s, :, :].bitcast(_f32r))
                arv = ar[:, :].rearrange("m (v f) -> m v f", f=5)  # [128, 503, 5]
                brv = br[:, :].rearrange("m (v f) -> m v f", f=5)

                # mean chain: bias_sb[:, t*4+kb] = -mu * colsum(W_t)[kb chunk]
                # (columns 4..2503 of slab A hold x exactly)
                colsum = stats.tile([128, 1], _f32, tag="colsum")
                nc.vector.reduce_sum(
                    out=colsum,
                    in_=ar[:, 4:2504].bitcast(_f32),
                    axis=mybir.AxisListType.X,
                )
                mups = mupsum.tile([1, 1], _f32, tag="mu")
                nc.tensor.matmul(
                    mups[:, :], lhsT=ones_col[:, :], rhs=colsum[:, :],
                    start=True, stop=True,
                )
                mu1 = stats.tile([1, 1], _f32, tag="mu1")
                nc.scalar.activation(
                    out=mu1, in_=mups[:, :],
                    func=mybir.ActivationFunctionType.Copy, scale=1.0 / L,
                )
                bcps = mupsum.tile([128, 1], _f32, tag="mu")
                nc.tensor.matmul(
                    bcps[:, :], lhsT=ones_row[:, :], rhs=mu1[:, :],
                    start=True, stop=True,
                )
                mu128 = stats.tile([128, 1], _f32, tag="mu128")
                nc.scalar.activation(
                    out=mu128, in_=bcps[:, :],
                    func=mybir.ActivationFunctionType.Copy, scale=1.0,
                )
                mui8 = stats.tile([8, 8], _f32, tag="mui8")
                nc.vector.tensor_scalar_mul(
                    out=mui8, in0=eye8_sb[:, :], scalar1=mu128[0:8, :]
                )
                bps = mupsum.tile([128, 8], _f32, tag="mu")
                nc.tensor.matmul(
                    bps[:, :], lhsT=nbars_sb[:, :], rhs=mui8[:, :],
                    start=True, stop=True,
                )
                bias_sb = stats.tile([128, 8], _f32, tag="bias")
                nc.scalar.activation(
                    out=bias_sb, in_=bps[:, :],
                    func=mybir.ActivationFunctionType.Copy, scale=1.0,
                )

                nyA = outs.tile([1, TPAD], _f32, tag="nyA")  # bin 0
                nyB = outs.tile([1, TPAD], _f32, tag="nyB")  # bin 512
                nyA_v = nyA[:, :].rearrange("m (v two) -> m v two", two=2)
                nyB_v = nyB[:, :].rearrange("m (v two) -> m v two", two=2)
                o_tiles = {}
                # sample 0: parity-major so all even-frame groups (slab A)
                # run before slab B / later weight halves arrive
                if s == 0:
                    group_iter = [(kb, par) for par in range(2) for kb in range(4)]
                else:
                    group_iter = [(kb, par) for kb in range(4) for par in range(2)]
                for kb, par in group_iter:
                    if kb not in o_tiles:
                        o_tiles[kb] = outs.tile([128, TPAD], _f32, tag="o", bufs=5, name=f"o_sb_{s}_{kb}")
                    o_sb = o_tiles[kb]
                    o_v = o_sb[:, :].rearrange("m (v two) -> m v two", two=2)
                    if True:
                        nf = NEV if par == 0 else NOD  # matmul free dim
                        nr = TEV if par == 0 else TOD  # real frames
                        pc = psums.tile([128, NEV], _f32, tag="pc", bufs=4)
                        ps_ = psums.tile([128, NEV], _f32, tag="ps", bufs=3)
                        for u in range(8):
                            if par == 0:
                                rhs = arv[:, u // 5 : u // 5 + nf, u % 5]
                            else:
                                c0 = (u + 2) // 5
                                rhs = brv[:, c0 : c0 + nf, (u + 2) % 5]
                            nc.tensor.matmul(
                                pc[:, :nf],
                                lhsT=wc_sb[:, u, 128 * kb : 128 * kb + 128],
                                rhs=rhs, start=(u == 0), stop=(u == 7),
                            )
                        for u in range(8):
                            if par == 0:
                                rhs = arv[:, u // 5 : u // 5 + nf, u % 5]
                            else:
                                c0 = (u + 2) // 5
                                rhs = brv[:, c0 : c0 + nf, (u + 2) % 5]
                            nc.tensor.matmul(
                                ps_[:, :nf],
                                lhsT=ws_sb[:, u, 128 * kb : 128 * kb + 128],
                                rhs=rhs, start=(u == 0), stop=(u == 7),
                            )
                        # power = (cos - mu*cbar)^2 + (sin - mu*sbar)^2
                        t1 = tmps.tile([128, NEV], _f32, tag="t1")
                        nc.scalar.activation(
                            out=t1[:, :nr], in_=pc[:, :nr],
                            func=mybir.ActivationFunctionType.Square,
                            bias=bias_sb[:, kb : kb + 1],
                        )
                        t2 = tmps.tile([128, NEV], _f32, tag="t2")
                        nc.scalar.activation(
                            out=t2[:, :nr], in_=ps_[:, :nr],
                            func=mybir.ActivationFunctionType.Square,
                            bias=bias_sb[:, 4 + kb : 5 + kb],
                        )
                        nc.vector.tensor_add(
                            out=t1[:, :nr], in0=t1[:, :nr], in1=t2[:, :nr]
                        )
                        nc.scalar.activation(
                            out=o_v[:, :nr, par], in_=t1[:, :nr],
                            func=mybir.ActivationFunctionType.Ln, bias=1.0,
                        )
                        if kb == 0:
                            # bin 0 (no sine) and bin 512 (Nyquist cosine,
                            # parked in sin-chunk partition 0): real-only.
                            f0 = tmps.tile([1, NEV], _f32, tag="f0")
                            nc.scalar.activation(
                                out=f0[:, :nr], in_=pc[0:1, :nr],
                                func=mybir.ActivationFunctionType.Square,
                                bias=bias_sb[0:1, 0:1],
                            )
                            nc.scalar.activation(
                                out=nyA_v[:, :nr, par], in_=f0[:, :nr],
                                func=mybir.ActivationFunctionType.Ln, bias=1.0,
                            )
                            f1 = tmps.tile([1, NEV], _f32, tag="f1")
                            nc.scalar.activation(
                                out=f1[:, :nr], in_=ps_[0:1, :nr],
                                func=mybir.ActivationFunctionType.Square,
                                bias=bias_sb[0:1, 4:5],
                            )
                            nc.scalar.activation(
                                out=nyB_v[:, :nr, par], in_=f1[:, :nr],
                                func=mybir.ActivationFunctionType.Ln, bias=1.0,
                            )
                    if par == 1:
                        if kb == 0:
                            nc.sync.dma_start(
                                out=o_d[s, 1:128, :], in_=o_sb[1:128, :T]
                            )
                        else:
                            nc.sync.dma_start(
                                out=o_d[s, 128 * kb : 128 * kb + 128, :],
                                in_=o_sb[:, :T],
                            )
                nc.sync.dma_start(out=o_d[s, 0:1, :], in_=nyA[:, :T])
                nc.sync.dma_start(out=o_d[s, 512:513, :], in_=nyB[:, :T])

    nc.compile()
    return nc


def _host_prepare(x, lambd):
    """Build per-core slab inputs + DFT matrices."""
    x = np.ascontiguousarray(x, dtype=np.float32)
    lam = float(np.asarray(lambd, dtype=np.float32))
    sq = np.sqrt(abs(lam)) if lam != 0 else 1.0

    n = np.arange(N_FFT, dtype=np.float64)
    win = 0.5 * (1.0 - np.cos(2.0 * np.pi * n / N_FFT))
    k = np.arange(512, dtype=np.float64)
    ang = 2.0 * np.pi * np.outer(n, k) / N_FFT
    wc64 = sq * win[:, None] * np.cos(ang)
    ws64 = -sq * win[:, None] * np.sin(ang)
    # sin k=0 column is all zeros; park the Nyquist cosine there
    ws64[:, 0] = sq * win * np.cos(np.pi * n)
    wc = np.ascontiguousarray(wc64.reshape(8, 128, 512).astype(np.float32))
    ws = np.ascontiguousarray(ws64.reshape(8, 128, 512).astype(np.float32))
    # negated per-bin column sums for the mean-correction bias
    # (column j of the bias matmul output: j = trig*4 + kb)
    nb = np.empty((8, 128), dtype=np.float64)
    for kb in range(4):
        nb[kb] = -wc64[:, 128 * kb : 128 * kb + 128].sum(axis=0)
        nb[4 + kb] = -ws64[:, 128 * kb : 128 * kb + 128].sum(axis=0)
    nbars = np.ascontiguousarray(nb.astype(np.float32))
    eye8 = np.eye(8, dtype=np.float32)

    # reflect pad + reshape: slab[m, q] = xp[128 q + m]
    xp = np.concatenate(
        [x[:, PAD:0:-1], x, x[:, L - 2 : L - 2 - PAD : -1]], axis=1
    )  # [B, LP]
    nq = 128 * QCOLS
    xa_f = np.zeros((B, nq), dtype=np.float32)
    xa_f[:, :LP] = xp
    xb_f = np.zeros((B, nq), dtype=np.float32)
    xb_f[:, : LP - 64] = xp[:, 64:]
    xa = np.ascontiguousarray(xa_f.reshape(B, QCOLS, 128).transpose(0, 2, 1))
    xb = np.ascontiguousarray(xb_f.reshape(B, QCOLS, 128).transpose(0, 2, 1))
    return xa, xb, wc, ws, nbars, eye8


def _in_maps(xa, xb, wc, ws, nbars, eye8):
    maps = []
    for c in range(NCORES):
        sl = slice(c * SPC, (c + 1) * SPC)
        maps.append(
            {
                "xa": np.ascontiguousarray(xa[sl]),
                "xb": np.ascontiguousarray(xb[sl]),
                "wc": wc,
                "ws": ws,
                "nbars": nbars,
                "eye8": eye8,
            }
        )
    return maps


def kernel(x, lambd):
    from concourse.bass_utils import run_bass_kernel_spmd

    prep = _host_prepare(x, lambd)
    nc = _build_module()
    res = run_bass_kernel_spmd(nc, _in_maps(*prep), core_ids=list(range(NCORES)))
    out = np.concatenate([res.results[c]["o"] for c in range(NCORES)], axis=0)
    return out.astype(np.float32)


if __name__ == "__main__":
    rng = np.random.default_rng(0)
    x = rng.standard_normal((B, L), dtype=np.float32)
    out = kernel(x, np.float32(5.0))
    print(out.shape, out.dtype, out[0, :3, :3])

